# revision 5
# baseline (speedup 1.0000x reference)
"""G-stationary f16 kernel.

y[b,h] = sum_{i,j} img[b,i]*txt[b,j]*W1r[i,j,h].  Fold img into the
stationary operand: G_i[j,b] = txt[b,j]*img[b,i] (DVE mult against a
host-replicated img tile), then PSUM accumulates over ALL (i,j) with no
per-i drain:  ps[b,h] += G_i[j,b].T @ W1[i,j,h].  f16 operands halve DMA
and LDWEIGHTS vs f32r; accuracy ~0.05% (tolerance 2e-2).

Startup/tail tuning: PE-warmup matmuls on scratch data hide the p-state
ramp inside the initial DMA wait; critical-path DMAs (txt, img0, W1[0])
issue first; W1 stream DMAs issue from the otherwise-idle Scalar queue;
partial-sum outputs ship as f16 pairs.
"""

import numpy as np
import ml_dtypes

import concourse.bass as bass
import concourse.tile as tile
from concourse import bacc, mybir
from concourse.bass_utils import run_bass_kernel_spmd

B, D, H = 512, 512, 512
N_CORES = 8
I_PER_CORE = D // N_CORES          # 64
N_BBLK = B // 128                  # 4
N_JCHUNK = D // 128                # 4
EPS = 1e-12
N_WARMUP = 16
# i-slabs computed in pure fp8e4m3 DoubleRow (2x MAC rate, ~4% rms noise
# on just these slabs -> 4.0%*sqrt(10/64) ~ 1.6% total, inside the 2e-2
# gate).  Spread in pairs so the DVE G-quantize load amortizes.
FP8_IS = (4, 5, 14, 15, 25, 26, 36, 37, 47, 48, 58, 59)
G8_SCALE = np.float32(2.0 ** 8)
W8_SCALE = np.float32(2.0 ** 9)
COMB = float(2.0 ** -17)

F32 = mybir.dt.float32
F16 = mybir.dt.float16
F8 = mybir.dt.float8e4
MULT = mybir.AluOpType.mult
ADD = mybir.AluOpType.add
DR = mybir.MatmulPerfMode.DoubleRow

_CACHE = {}


def _l2norm(x: np.ndarray) -> np.ndarray:
    n = np.sqrt(np.sum(x * x, axis=1, keepdims=True, dtype=np.float32))
    return (x / np.maximum(n, np.float32(EPS))).astype(np.float32)


def build_nc():
    nc = bacc.Bacc(
        "TRN2",
        target_bir_lowering=False,
        debug=False,
        num_devices=N_CORES,
    )

    # txt packed [p, c, b] so one DMA loads all 4 j-chunks with 4KB lines
    txt_t = nc.dram_tensor("txt_t", [128, N_JCHUNK, B], F16, kind="ExternalInput").ap()
    # img replicated across partitions, 4 i-values per row (4KB DMA lines)
    imgr = nc.dram_tensor(
        "imgr", [I_PER_CORE // 4, 128, 4 * B], F16, kind="ExternalInput"
    ).ap()
    # W1 packed per-i as [128 partitions, 4 j-chunks * H] (4KB DMA lines)
    w1_s = nc.dram_tensor(
        "w1_s", [I_PER_CORE, 128, N_JCHUNK * H], F16, kind="ExternalInput"
    ).ap()
    # [half, p, sub, H]: b = half*256 + sub*128 + p
    # fp8 stream: W1*2^9 as [idx][p][c2][t][h]; img*2^8 replicated in pairs
    w1_8 = nc.dram_tensor(
        "w1_8", [len(FP8_IS), 128, 2, 2, H], F8, kind="ExternalInput"
    ).ap()
    imgr8 = nc.dram_tensor(
        "imgr8", [len(FP8_IS) // 2, 128, 2, B], F16, kind="ExternalInput"
    ).ap()
    yp = nc.dram_tensor("yp", [2, 128, 2, H], F16, kind="ExternalOutput").ap()

    with tile.TileContext(nc) as tc:
        with (
            tc.tile_pool(name="const", bufs=1) as constp,
            tc.tile_pool(name="w1", bufs=6) as w1p,
            tc.tile_pool(name="imgp", bufs=3) as imgp,
            tc.tile_pool(name="g", bufs=2) as gp,
            tc.tile_pool(name="g8", bufs=2) as g8p,
            tc.tile_pool(name="w18", bufs=2) as w18p,
            tc.tile_pool(name="img8", bufs=2) as img8p,
            tc.tile_pool(name="ps", bufs=1, space=bass.MemorySpace.PSUM) as psump,
        ):
            # PE warmup: garbage matmuls on scratch data ramp the PE to max
            # p-state while the first real DMAs are in flight.  Scratch is
            # memset on GpSimd, whose preamble finishes earliest.
            scratch = constp.tile([128, H], F16, tag="scratch", name="scratch")
            ps8 = [
                psump.tile([128, H], F32, tag=f"ps8_{bb}", name=f"ps8_{bb}")
                for bb in range(N_BBLK)
            ]
            wps = ps8[0]
            nc.gpsimd.memset(scratch[:], 0.0)
            for _ in range(N_WARMUP):
                nc.tensor.matmul(
                    wps[:], scratch[:, :128], scratch[:], start=True, stop=True
                )

            # Critical-path loads, smallest first so tile deps release early:
            # Sync queue: txt c0/c1, img i=0 strip, txt c2/c3, img block 0...
            # Scalar queue: W1 i=0 in four chunk tiles, then full i-slabs.
            txt01 = constp.tile([128, 2, B], F16, tag="txt01", name="txt01")
            nc.sync.dma_start(txt01[:], txt_t[:, 0:2])
            img0 = constp.tile([128, B], F16, tag="img0", name="img0")
            nc.sync.dma_start(img0[:], imgr[0][:, 0:B])
            txt23 = constp.tile([128, 2, B], F16, tag="txt23", name="txt23")
            nc.sync.dma_start(txt23[:], txt_t[:, 2:4])
            img_first = imgp.tile([128, 4 * B], F16, tag="imgr", name="imgr")
            nc.sync.dma_start(img_first[:], imgr[0])
            # i=0 and i=1 W1 slabs arrive as four chunk tiles each so the
            # stream can start on chunk 0 and never stall across i=0 -> i=1.
            w1t0c = []
            for i0 in range(2):
                for c in range(N_JCHUNK):
                    t = constp.tile(
                        [128, H], F16, tag=f"w1t{i0}c{c}", name=f"w1t{i0}c{c}"
                    )
                    nc.scalar.dma_start(t[:], w1_s[i0][:, c * H : (c + 1) * H])
                    w1t0c.append(t)

            def txt_ap(c):
                return txt01[:, c] if c < 2 else txt23[:, c - 2]

            ps = [
                psump.tile([128, H], F32, tag=f"ps{bb}", name=f"ps{bb}")
                for bb in range(N_BBLK)
            ]

            it = None
            for i in range(I_PER_CORE):
                if i % 4 == 0 and i > 0:
                    it = imgp.tile([128, 4 * B], F16, tag="imgr", name="imgr")
                    nc.sync.dma_start(it[:], imgr[i // 4])
                if i > 1 and i not in FP8_IS:
                    w1t = w1p.tile([128, N_JCHUNK * H], F16, tag="w1", name="w1")
                    nc.scalar.dma_start(w1t[:], w1_s[i])
                io = (i % 4) * B
                if i in FP8_IS:
                    idx = FP8_IS.index(i)
                    if idx % 2 == 0:
                        i8t = img8p.tile([128, 2, B], F16, tag="i8", name="i8")
                        nc.sync.dma_start(i8t[:], imgr8[idx // 2])
                    w18t = w18p.tile([128, 2, 2, H], F8, tag="w18", name="w18")
                    nc.scalar.dma_start(w18t[:], w1_8[idx])
                    g8 = [
                        g8p.tile([128, 2, B], F8, tag=f"g8{c2}", name=f"g8{c2}")
                        for c2 in range(2)
                    ]
                    for c2 in range(2):
                        for tt in range(2):
                            nc.vector.tensor_mul(
                                g8[c2][:, tt], txt_ap(2 * c2 + tt), i8t[:, idx % 2]
                            )
                    for bb in range(N_BBLK):
                        bs = slice(bb * 128, (bb + 1) * 128)
                        for c2 in range(2):
                            nc.tensor.matmul(
                                ps8[bb][:],
                                g8[c2][:, :, bs],
                                w18t[:, c2],
                                start=(i == FP8_IS[0] and c2 == 0),
                                stop=(i == FP8_IS[-1] and c2 == 1),
                                perf_mode=DR,
                            )
                    continue
                g = [
                    gp.tile([128, B], F16, tag=f"g{c}", name=f"g{c}")
                    for c in range(N_JCHUNK)
                ]
                if i > 1:
                    for c in range(N_JCHUNK):
                        nc.vector.tensor_mul(
                            g[c][:], txt_ap(c), it[:, io : io + B]
                        )
                if i <= 1:
                    # c-outer: the first matmuls need only txt c0 + W1 chunk 0
                    if i == 0:
                        it = img_first
                        for c in range(N_JCHUNK):
                            nc.vector.tensor_mul(g[c][:], txt_ap(c), img0[:])
                    else:
                        for c in range(N_JCHUNK):
                            nc.vector.tensor_mul(
                                g[c][:], txt_ap(c), it[:, io : io + B]
                            )
                    for c in range(N_JCHUNK):
                        for bb in range(N_BBLK):
                            nc.tensor.matmul(
                                ps[bb][:],
                                g[c][:, bb * 128 : (bb + 1) * 128],
                                w1t0c[i * N_JCHUNK + c][:],
                                start=(i == 0 and c == 0),
                                stop=False,
                            )
                else:
                    for bb in range(N_BBLK):
                        for c in range(N_JCHUNK):
                            nc.tensor.matmul(
                                ps[bb][:],
                                g[c][:, bb * 128 : (bb + 1) * 128],
                                w1t[:, c * H : (c + 1) * H],
                                start=False,
                                stop=(i == I_PER_CORE - 1 and c == N_JCHUNK - 1),
                            )

            for bb in range(N_BBLK):
                o = constp.tile([128, H], F16, tag=f"o{bb}", name=f"o{bb}")
                nc.scalar.copy(o[:], ps[bb][:])
                nc.vector.scalar_tensor_tensor(
                    o[:], ps8[bb][:], COMB, o[:], op0=MULT, op1=ADD
                )
                nc.sync.dma_start(yp[bb // 2, :, bb % 2], o[:])

    nc.compile()
    return nc


def make_in_maps(image_embeds, text_embeds, W1):
    imgn = _l2norm(np.asarray(image_embeds, np.float32))
    txtn = _l2norm(np.asarray(text_embeds, np.float32))
    # [c, p, b] -> [p, c, b]
    txt_t = np.ascontiguousarray(
        txtn.T.reshape(N_JCHUNK, 128, B).transpose(1, 0, 2).astype(np.float16)
    )
    W1r = np.asarray(W1, np.float32).reshape(D, D, H)
    in_maps = []
    for c in range(N_CORES):
        sl = slice(c * I_PER_CORE, (c + 1) * I_PER_CORE)
        # [64, 4c, 128p, H] -> [64, 128p, 4c, H]: per-i DMA gets 4KB lines
        w1c = np.ascontiguousarray(
            W1r[sl]
            .reshape(I_PER_CORE, N_JCHUNK, 128, H)
            .transpose(0, 2, 1, 3)
            .reshape(I_PER_CORE, 128, N_JCHUNK * H)
            .astype(np.float16)
        )
        img_slice = imgn[:, sl].T.astype(np.float16)           # [64, B]
        imgr = np.ascontiguousarray(
            np.broadcast_to(
                img_slice.reshape(I_PER_CORE // 4, 1, 4 * B),
                (I_PER_CORE // 4, 128, 4 * B),
            )
        )
        fp8_list = list(FP8_IS)
        Wc8 = (W1r[sl][fp8_list] * W8_SCALE).astype(np.float32)
        w1_8 = np.ascontiguousarray(
            Wc8.reshape(len(fp8_list), 2, 2, 128, H)
            .transpose(0, 3, 1, 2, 4)
            .astype(ml_dtypes.float8_e4m3fn)
        )
        img8 = (img_slice.astype(np.float32)[fp8_list] * G8_SCALE).astype(
            np.float16
        )                                                   # [10, B]
        imgr8 = np.ascontiguousarray(
            np.broadcast_to(
                img8.reshape(len(fp8_list) // 2, 1, 2, B),
                (len(fp8_list) // 2, 128, 2, B),
            )
        )
        in_maps.append(
            {"txt_t": txt_t, "imgr": imgr, "w1_s": w1c, "w1_8": w1_8,
             "imgr8": imgr8}
        )
    return in_maps


def run_device(in_maps, trace=False, **kw):
    if "nc" not in _CACHE:
        _CACHE["nc"] = build_nc()
    return run_bass_kernel_spmd(
        _CACHE["nc"], in_maps, list(range(N_CORES)), trace=trace, **kw
    )


def finish_host(results, b1, W2, b2):
    Y = np.zeros((B, H), np.float32)
    for c in range(N_CORES):
        # [half, p, sub, H] -> [half, sub, p, H] -> [B, H]
        Y += (
            results[c]["yp"]
            .astype(np.float32)
            .transpose(0, 2, 1, 3)
            .reshape(B, H)
        )
    h = np.maximum(Y + np.asarray(b1, np.float32), np.float32(0.0))
    out = h @ np.asarray(W2, np.float32) + np.asarray(b2, np.float32)
    return out.astype(np.float32)


def kernel(image_embeds, text_embeds, W1, b1, W2, b2):
    in_maps = make_in_maps(image_embeds, text_embeds, W1)
    res = run_device(in_maps, trace=False)
    return finish_host(res.results, b1, W2, b2)
